# revision 17
# baseline (speedup 1.0000x reference)
"""Trainium2 Bass kernel for nn_MMHA_78039555768536.

Gated mix of per-segment causal softmax attention and a linear-attention
memory (delta rule, memory summed over batch per segment).

Strategy (8 cores): reformulate the memory recurrence as a linear matrix
recurrence  M_{t+1} = A_t M_t + B_t  with
    A_t = I - sum_b sk_b^T diag(1/d_b) sk_b   (symmetric A-part)
    B_t = sum_b sk_b^T v_b
    d_b = sk_b @ z_{b,t};  z is a prefix of column-sums of sk (M-independent)
Core c owns segments {2c, 2c+1} for all batches.  Two all-gathers:
 AG1: per-segment colsums of sk (for the z prefix)  [tiny]
 AG2: per-core pair composition (Abar^T, Bbar)      [1 MB bf16 per rank]
Then every core redundantly runs the 7-step pair chain and selects its own
prefix M via a per-core one-hot input (SPMD, no branches).

v2 schedule: all work that feeds AG2 (k-proj, colsums, AG1, z, v-proj,
A/B, compose) runs first; AG2 is issued, and the whole attention block
(q-proj, scores, softmax, attn@v, sq=elu1(q), memory-read denominators)
executes while the collective is in flight.  k is projected once
(transposed form) and PE-transposed into s-major for sk.  All partition
broadcasts are SBUF->SBUF gpsimd ops (no DRAM round-trips).
"""

import os
import sys

sys.path.insert(0, "/opt/trn_rl_repo")

STAGE = int(os.environ.get("KSTAGE", "9"))
SIMSAFE = int(os.environ.get("KSIMSAFE", "0"))  # CoreSim rejects PSUM re-accumulate

from contextlib import ExitStack

import numpy as np
import ml_dtypes

import concourse.bass as bass
import concourse.bacc as bacc
import concourse.tile as tile
from concourse import mybir
from concourse import bass_utils

B, L, DIN = 4, 8192, 512
H, D, SEG = 8, 64, 512
HD = H * D
NSEG = L // SEG          # 16
NC = 8                   # cores
SPC = NSEG // NC         # segments per core = 2
P = 128
NB = HD // P             # 4 blocks of 128
BS = B * SPC             # batch-segment units per core = 8

bf = mybir.dt.bfloat16
f32 = mybir.dt.float32
AF = mybir.ActivationFunctionType
OP = mybir.AluOpType
bf_np = ml_dtypes.bfloat16

_CACHE = {}


def _build():
    nc = bacc.Bacc(
        "TRN2",
        target_bir_lowering=False,
        debug=False,
        enable_asserts=False,
        num_devices=NC,
    )

    # ---------------- DRAM I/O ----------------
    xt_d = nc.dram_tensor("xt", [B, SPC, NB, P, SEG], bf, kind="ExternalInput").ap()
    wq_d = nc.dram_tensor("wq", [NB, P, HD], bf, kind="ExternalInput").ap()
    wk_d = nc.dram_tensor("wk", [NB, P, HD], bf, kind="ExternalInput").ap()
    wv_d = nc.dram_tensor("wv", [NB, P, HD], bf, kind="ExternalInput").ap()
    wd_d = nc.dram_tensor("wd", [NB, P, D], bf, kind="ExternalInput").ap()
    gcol_d = nc.dram_tensor("gcol", [P, NB], f32, kind="ExternalInput").ap()
    omg_d = nc.dram_tensor("omg", [P, NB], f32, kind="ExternalInput").ap()
    zmask_d = nc.dram_tensor("zmask", [64, NC], f32, kind="ExternalInput").ap()
    oh_d = nc.dram_tensor("oh", [P, NC], f32, kind="ExternalInput").ap()
    mask_d = nc.dram_tensor("cmask", [P, P], bf, kind="ExternalInput").ap()
    ident_d = nc.dram_tensor("ident", [P, P], bf, kind="ExternalInput").ap()
    out_d = nc.dram_tensor("out", [B, SPC, SEG, D], f32, kind="ExternalOutput").ap()

    with tile.TileContext(nc) as tc, ExitStack() as ctx:
        # ---------------- constant / DRAM pools ----------------
        const = ctx.enter_context(tc.tile_pool(name="const", bufs=1))
        dram = ctx.enter_context(tc.tile_pool(name="dram", bufs=1, space="DRAM"))
        keep = ctx.enter_context(tc.tile_pool(name="keep", bufs=BS))
        phb = ctx.enter_context(tc.tile_pool(name="phb", bufs=1))  # phase singles

        WQ = const.tile([P, NB, HD], bf)
        WK = const.tile([P, NB, HD], bf)
        WV = const.tile([P, NB, HD], bf)
        WD = const.tile([P, NB, D], bf)
        GC = const.tile([P, NB], f32)
        OMG = const.tile([P, NB], f32)
        ZM = const.tile([64, NC], f32)
        OH = const.tile([P, NC], f32)
        CM = const.tile([P, P], bf)
        ID = const.tile([P, P], bf)
        ONE = const.tile([P, 1], bf)
        ONER = const.tile([1, P], bf)   # ones row (PE partition-broadcast)

        nc.sync.dma_start(out=WQ, in_=wq_d.rearrange("kb p n -> p kb n"))
        nc.sync.dma_start(out=WK, in_=wk_d.rearrange("kb p n -> p kb n"))
        nc.sync.dma_start(out=WV, in_=wv_d.rearrange("kb p n -> p kb n"))
        nc.sync.dma_start(out=WD, in_=wd_d.rearrange("kb p n -> p kb n"))
        nc.sync.dma_start(out=GC, in_=gcol_d)
        nc.sync.dma_start(out=OMG, in_=omg_d)
        nc.sync.dma_start(out=ZM, in_=zmask_d)
        nc.sync.dma_start(out=OH, in_=oh_d)
        nc.sync.dma_start(out=CM, in_=mask_d)
        nc.sync.dma_start(out=ID, in_=ident_d)
        nc.vector.memset(ONE, 1.0)
        nc.vector.memset(ONER, 1.0)

        # collective bounce buffers
        cs_in = dram.tile([BS, HD], f32)
        cs_out = dram.tile([NC * BS, HD], f32)
        ab_in = dram.tile([2, HD, HD], bf)
        ab_out = dram.tile([NC, 2, HD, HD], bf, addr_space="Shared")
        step_d = dram.tile([BS, NB, P, SEG], bf)  # attention-term scratch

        # retained across phases (bufs=BS -> one slot per batch-segment)
        khT = [keep.tile([P, NB, SEG], bf, tag="kh", name=f"kh{i}") for i in range(BS)]
        skT = [keep.tile([P, NB, HD], bf, tag="sk", name=f"sk{i}") for i in range(BS)]
        sqT = skT   # sk dies before sq is born; share the slots
        vaT = [keep.tile([P, NB, H, D + 1], bf, tag="va", name=f"va{i}")
               for i in range(BS)]
        rcmT = [keep.tile([1, SEG], bf, tag="rcm", name=f"rcm{i}") for i in range(BS)]
        rcmbT = [keep.tile([P, SEG], bf, tag="rcmb", name=f"rcmb{i}")
                 for i in range(BS)]

        # phase singles
        ZCOL = phb.tile([P, NB, BS], bf)    # column form for denominators
        AT0 = phb.tile([P, NB, HD], bf)     # segment-0 A-part (retained)
        BT0 = phb.tile([P, NB, HD], bf)
        MSEL = phb.tile([P, NB, HD], bf)    # selected M at segment 2c
        MLOC1 = phb.tile([P, NB, HD], bf)   # M at segment 2c+1

        def bs_of(b, j):
            return j * B + b

        # ============ PHASE A1: k-proj (transposed), transpose->sk, colsums ====
        with tc.tile_pool(name="pa1", bufs=2) as pa1, \
             tc.tile_pool(name="ps1", bufs=2, space="PSUM") as ps1, \
             tc.tile_pool(name="pst", bufs=2, space="PSUM") as pst:
            for j in range(SPC):
                for b in range(B):
                    i = bs_of(b, j)
                    XT = pa1.tile([P, NB, SEG], bf, tag="xt")
                    nc.sync.dma_start(out=XT, in_=xt_d[b, j].rearrange("kb p s -> p kb s"))
                    kh_i = khT[i]
                    for mb in range(NB):
                        pk = ps1.tile([P, SEG], f32, tag="pk")
                        for kb in range(NB):
                            nc.tensor.matmul(
                                pk, lhsT=WK[:, kb, mb * P:(mb + 1) * P],
                                rhs=XT[:, kb, :],
                                start=(kb == 0), stop=(kb == NB - 1),
                            )
                        nc.scalar.activation(kh_i[:, mb, :], pk, AF.Copy)
                    # transpose kh -> s-major, fused elu1 into sk
                    sk_i = skT[i]
                    for sb in range(NB):
                        pt = pst.tile([P, HD], f32, tag="pt")
                        for mb in range(NB):
                            nc.tensor.matmul(
                                pt[:, mb * P:(mb + 1) * P],
                                lhsT=kh_i[:, mb, sb * P:(sb + 1) * P],
                                rhs=ID, start=True, stop=True,
                            )
                        # elu1(k) = max(k + 1, exp(min(k, 0)))
                        em = pa1.tile([P, HD], bf, tag="em")
                        nc.vector.tensor_scalar_min(em, pt, 0.0)
                        ee = pa1.tile([P, HD], bf, tag="ee")
                        nc.scalar.activation(ee, em, AF.Exp)
                        nc.vector.scalar_tensor_tensor(
                            out=sk_i[:, sb, :], in0=pt, scalar=1.0, in1=ee,
                            op0=OP.add, op1=OP.max,
                        )
                    pc = ps1.tile([1, HD], f32, tag="pc")
                    for sb in range(NB):
                        nc.tensor.matmul(
                            pc, lhsT=ONE, rhs=sk_i[:, sb, :],
                            start=(sb == 0), stop=(sb == NB - 1),
                        )
                    cs_sb = pa1.tile([1, HD], f32, tag="cs")
                    nc.scalar.activation(cs_sb, pc, AF.Copy)
                    nc.sync.dma_start(out=cs_in[i:i + 1, :], in_=cs_sb)

        # ============ AG1: colsums ============
        nc.gpsimd.collective_compute(
            "AllGather", OP.bypass,
            replica_groups=[list(range(NC))],
            ins=[cs_in.opt()], outs=[cs_out.opt()],
        )

        # ============ v-proj (all units) + z prefix + A/B + compose ============
        with tc.tile_pool(name="pz", bufs=1) as pz, \
             tc.tile_pool(name="psz", bufs=1, space="PSUM") as psz, \
             tc.tile_pool(name="pskd", bufs=B) as pskd, \
             tc.tile_pool(name="pv2", bufs=2) as pv2, \
             tc.tile_pool(name="pab", bufs=1) as pab, \
             tc.tile_pool(name="ps2", bufs=2, space="PSUM") as ps2:
            # v-proj for all 8 units first: no dependency on AG1, so the PE
            # queue is not blocked behind the collective.
            for j in range(SPC):
                for b in range(B):
                    i = bs_of(b, j)
                    XT = pv2.tile([P, NB, SEG], bf, tag="xt")
                    nc.sync.dma_start(out=XT, in_=xt_d[b, j].rearrange("kb p s -> p kb s"))
                    va = vaT[i]
                    nc.vector.memset(va[:, :, :, D:D + 1], 1.0)
                    for sb in range(NB):
                        pv = ps2.tile([P, SEG], f32, tag="pp")
                        for kb in range(NB):
                            nc.tensor.matmul(
                                pv, lhsT=XT[:, kb, sb * P:(sb + 1) * P],
                                rhs=WV[:, kb, :],
                                start=(kb == 0), stop=(kb == NB - 1),
                            )
                        nc.vector.tensor_copy(
                            va[:, sb, :, 0:D], pv.rearrange("p (h d) -> p h d", h=H)
                        )

            Z = pz.tile([NC * BS, HD], f32, tag="z")
            nc.sync.dma_start(out=Z, in_=cs_out)
            # per-unit z rows, each in its own partition-0 tile (ISA
            # partition_broadcast requires an aligned source partition)
            zrow16 = []
            for i in range(BS):
                zpi = psz.tile([1, HD], f32, tag="zpi", bufs=2)
                nc.tensor.matmul(zpi, lhsT=ZM[:, i:i + 1], rhs=Z,
                                 start=True, stop=True)
                zr = pz.tile([1, HD], bf, tag="zr16", name=f"zr16_{i}", bufs=BS)
                nc.scalar.activation(zr, zpi, AF.Copy, bias=1.0 / D)
                zrow16.append(zr)
            for kb in range(NB):
                zc = psz.tile([P, BS], f32, tag="zc")
                nc.tensor.matmul(zc, lhsT=Z[:, kb * P:(kb + 1) * P], rhs=ZM,
                                 start=True, stop=True)
                nc.scalar.activation(ZCOL[:, kb, :], zc, AF.Copy, bias=1.0 / D)

            at1 = bt1 = None
            for j in range(SPC):
                skd = [None] * B
                for b in range(B):
                    i = bs_of(b, j)
                    # --- d and sk/d (elementwise on gpsimd, free pre-AG2) ---
                    sk_i = skT[i]
                    sd = pskd.tile([P, NB, HD], bf, tag="skd")
                    skd[b] = sd
                    dcol = pv2.tile([P, NB], f32, tag="d")
                    rcd = pv2.tile([P, NB], f32, tag="rcd")
                    zbp = pv2.tile([P, HD], bf, tag="zbp")
                    nc.gpsimd.partition_broadcast(zbp, zrow16[i])
                    for sb in range(NB):
                        jnk = pv2.tile([P, HD], bf, tag="jnk")
                        nc.gpsimd.tensor_mul(jnk, sk_i[:, sb, :], zbp)
                        nc.vector.tensor_reduce(
                            out=dcol[:, sb:sb + 1], in_=jnk,
                            axis=mybir.AxisListType.X, op=OP.add,
                        )
                    nc.vector.reciprocal(rcd, dcol)
                    for sb in range(NB):
                        nc.vector.tensor_scalar_mul(
                            sd[:, sb, :], sk_i[:, sb, :], rcd[:, sb:sb + 1]
                        )

                # --- A_t, B_t for this segment (sum over batches) ---
                at_t = pab.tile([P, NB, HD], bf, tag="at", name=f"at{j}") if j > 0 else AT0
                bt_t = pab.tile([P, NB, HD], bf, tag="bt", name=f"bt{j}") if j > 0 else BT0
                for mb in range(NB):
                    pA = ps2.tile([P, HD], f32, tag="pp")
                    n = 0
                    for b in range(B):
                        for sb in range(NB):
                            nc.tensor.matmul(
                                pA,
                                lhsT=skT[bs_of(b, j)][:, sb, mb * P:(mb + 1) * P],
                                rhs=skd[b][:, sb, :],
                                start=(n == 0), stop=(n == B * NB - 1),
                            )
                            n += 1
                    # negate: A-part = -K
                    nc.scalar.activation(at_t[:, mb, :], pA, AF.Copy, scale=-1.0)
                for mb in range(NB):
                    pB = ps2.tile([P, HD], f32, tag="pp")
                    n = 0
                    for b in range(B):
                        for sb in range(NB):
                            nc.tensor.matmul(
                                pB.rearrange("p (h d) -> p h d", h=H),
                                lhsT=skT[bs_of(b, j)][:, sb, mb * P:(mb + 1) * P],
                                rhs=vaT[bs_of(b, j)][:, sb, :, 0:D],
                                start=(n == 0), stop=(n == B * NB - 1),
                            )
                            n += 1
                    nc.scalar.activation(bt_t[:, mb, :], pB, AF.Copy)
                if j > 0:
                    at1, bt1 = at_t, bt_t

            # --- pair composition: abA = Abar^T = A0 A1 + A0 + A1 ; abB = Bbar ---
            abA = pab.tile([P, NB, HD], bf, tag="abA")
            abB = pab.tile([P, NB, HD], bf, tag="abB")
            for mb in range(NB):
                pA = ps2.tile([P, HD], f32, tag="pp")
                for kb in range(NB):
                    nc.tensor.matmul(
                        pA, lhsT=AT0[:, kb, mb * P:(mb + 1) * P], rhs=at1[:, kb, :],
                        start=(kb == 0), stop=False,
                    )
                nc.tensor.matmul(pA, lhsT=ID, rhs=AT0[:, mb, :], start=False, stop=False)
                nc.tensor.matmul(pA, lhsT=ID, rhs=at1[:, mb, :], start=False, stop=True)
                nc.scalar.activation(abA[:, mb, :], pA, AF.Copy)
            for mb in range(NB):
                pB = ps2.tile([P, HD], f32, tag="pp")
                for kb in range(NB):
                    nc.tensor.matmul(
                        pB, lhsT=at1[:, kb, mb * P:(mb + 1) * P], rhs=BT0[:, kb, :],
                        start=(kb == 0), stop=False,
                    )
                nc.tensor.matmul(pB, lhsT=ID, rhs=BT0[:, mb, :], start=False, stop=False)
                nc.tensor.matmul(pB, lhsT=ID, rhs=bt1[:, mb, :], start=False, stop=True)
                nc.scalar.activation(abB[:, mb, :], pB, AF.Copy)
            nc.sync.dma_start(out=ab_in[0].rearrange("(kb p) n -> p kb n", p=P), in_=abA)
            nc.sync.dma_start(out=ab_in[1].rearrange("(kb p) n -> p kb n", p=P), in_=abB)

        # ============ AG2: pair compositions (overlapped with attention) ======
        nc.gpsimd.collective_compute(
            "AllGather", OP.bypass,
            replica_groups=[list(range(NC))],
            ins=[ab_in.opt()], outs=[ab_out.opt()],
        )

        if STAGE >= 2:
            # ============ WINDOW: q-proj, sq, attention, mem-read denominators
            with tc.tile_pool(name="pw1", bufs=2) as pw1, \
                 tc.tile_pool(name="pw", bufs=3) as pw, \
                 tc.tile_pool(name="psq", bufs=2, space="PSUM") as psq, \
                 tc.tile_pool(name="psc", bufs=2, space="PSUM") as psc, \
                 tc.tile_pool(name="psa", bufs=2, space="PSUM") as psa, \
                 tc.tile_pool(name="psd", bufs=1, space="PSUM") as psd:
                for j in range(SPC):
                    for b in range(B):
                        i = bs_of(b, j)
                        XT = pw1.tile([P, NB, SEG], bf, tag="xt")
                        nc.sync.dma_start(out=XT, in_=xt_d[b, j].rearrange("kb p s -> p kb s"))
                        # --- qT (hd on partitions) + sq = elu1(q) ---
                        qh = pw1.tile([P, NB, SEG], bf, tag="qh")
                        sq_i = sqT[i]
                        for mb in range(NB):
                            pq = psq.tile([P, SEG], f32, tag="pp")
                            for kb in range(NB):
                                nc.tensor.matmul(
                                    pq, lhsT=WQ[:, kb, mb * P:(mb + 1) * P],
                                    rhs=XT[:, kb, :],
                                    start=(kb == 0), stop=(kb == NB - 1),
                                )
                            nc.scalar.activation(qh[:, mb, :], pq, AF.Copy)
                            em = pw1.tile([P, SEG], bf, tag="em")
                            nc.vector.tensor_scalar_min(em, pq, 0.0)
                            ee = pw1.tile([P, SEG], bf, tag="ee")
                            nc.scalar.activation(ee, em, AF.Exp)
                            nc.vector.scalar_tensor_tensor(
                                out=sq_i[:, mb, :], in0=pq, scalar=1.0, in1=ee,
                                op0=OP.add, op1=OP.max,
                            )
                        # --- memory-read denominator (broadcast happens post-AG2) ---
                        pd = psd.tile([1, SEG], f32, tag="dn")
                        for kb in range(NB):
                            nc.tensor.matmul(
                                pd, lhsT=ZCOL[:, kb, i:i + 1], rhs=sq_i[:, kb, :],
                                start=(kb == 0), stop=(kb == NB - 1),
                            )
                        with nc.allow_low_precision(reason="bf16 memread recip"):
                            nc.vector.reciprocal(rcmT[i], pd)

                        # --- attention (no Pool-engine ops: queue is blocked
                        #     behind AG2; broadcast reciprocals via PE) ---
                        kh_i = khT[i]
                        va = vaT[i]
                        st_i = pw1.tile([P, NB, SEG], bf, tag="stp")
                        for h in range(H):
                            hb, ho = h // 2, (h % 2) * 64
                            pat = psa.tile([D + 1, SEG], f32, tag="at")
                            for kb in range(NB):
                                q0 = kb * P
                                qf = SEG - q0
                                ps_ = psc.tile([P, SEG], f32, tag="sc")
                                nc.tensor.matmul(
                                    ps_[:, 0:qf],
                                    lhsT=kh_i[ho:ho + 64, hb, q0:q0 + P],
                                    rhs=qh[ho:ho + 64, hb, q0:SEG],
                                    start=True, stop=True,
                                )
                                wt = pw.tile([P, SEG], bf, tag="wt")
                                nc.scalar.activation(wt[:, 0:qf], ps_[:, 0:qf], AF.Exp,
                                                     scale=0.125)
                                # causal mask on the diagonal 128x128 block
                                nc.vector.tensor_mul(wt[:, 0:P], wt[:, 0:P], CM)
                                nc.tensor.matmul(
                                    pat[:, q0:SEG],
                                    lhsT=va[:, kb, h, :],
                                    rhs=wt[:, 0:qf],
                                    start=(kb == 0), stop=(kb == NB - 1),
                                )
                            rca = pw.tile([1, SEG], bf, tag="rca")
                            with nc.allow_low_precision(reason="bf16 softmax recip"):
                                nc.vector.reciprocal(rca, pat[D:D + 1, :])
                            # broadcast 1/den across partitions via PE (Pool
                            # queue is blocked behind AG2)
                            pbc = psd.tile([D, SEG], f32, tag="bc")
                            nc.tensor.matmul(pbc, lhsT=ONER[:, 0:D], rhs=rca,
                                             start=True, stop=True)
                            # DVE may read only one PSUM operand: move the
                            # numerator to SBUF (folding the (1-g) gate) first
                            stp = pw.tile([D, SEG], bf, tag="stpre")
                            nc.scalar.activation(stp, pat[0:D, :], AF.Copy,
                                                 scale=OMG[ho:ho + 64, hb:hb + 1])
                            nc.vector.tensor_mul(st_i[ho:ho + 64, hb, :], stp, pbc)
                        nc.sync.dma_start(
                            out=step_d[i].rearrange("kb p s -> p kb s"), in_=st_i)

        if STAGE >= 3:
            # ============ chain + select ============
            # Pool queue is free again after AG2: broadcast the memory-read
            # reciprocals for phase B.
            for i in range(BS):
                nc.gpsimd.partition_broadcast(rcmbT[i], rcmT[i])
            nc.vector.memset(MSEL, 0.0)
            with tc.tile_pool(name="pch", bufs=2) as pch, \
                 tc.tile_pool(name="psch", bufs=NB, space="PSUM") as psch:
                pM = [psch.tile([P, HD], f32, tag="ch", name=f"chain{i}") for i in range(NB)]
                mprev = None
                for step in range(NC - 1):
                    cA = pch.tile([P, NB, HD], bf, tag="cA")
                    cB = pch.tile([P, NB, HD], bf, tag="cB")
                    nc.sync.dma_start(
                        out=cA, in_=ab_out[step, 0].rearrange("(kb p) n -> p kb n", p=P))
                    nc.sync.dma_start(
                        out=cB, in_=ab_out[step, 1].rearrange("(kb p) n -> p kb n", p=P))
                    mcur = pch.tile([P, NB, HD], bf, tag="mc")
                    for mb in range(NB):
                        if step == 0:
                            nc.tensor.matmul(pM[mb], lhsT=ID, rhs=cB[:, mb, :],
                                             start=True, stop=True)
                        elif SIMSAFE:
                            for kb in range(NB):
                                nc.tensor.matmul(
                                    pM[mb], lhsT=cA[:, kb, mb * P:(mb + 1) * P],
                                    rhs=mprev[:, kb, :],
                                    start=(kb == 0), stop=False,
                                )
                            nc.tensor.matmul(pM[mb], lhsT=ID, rhs=mprev[:, mb, :],
                                             start=False, stop=False)
                            nc.tensor.matmul(pM[mb], lhsT=ID, rhs=cB[:, mb, :],
                                             start=False, stop=True)
                        else:
                            for kb in range(NB):
                                nc.tensor.matmul(
                                    pM[mb], lhsT=cA[:, kb, mb * P:(mb + 1) * P],
                                    rhs=mprev[:, kb, :],
                                    start=False, stop=False,
                                )
                            nc.tensor.matmul(pM[mb], lhsT=ID, rhs=cB[:, mb, :],
                                             start=False, stop=True)
                        nc.scalar.activation(mcur[:, mb, :], pM[mb], AF.Copy)
                        nc.vector.scalar_tensor_tensor(
                            out=MSEL[:, mb, :], in0=mcur[:, mb, :],
                            scalar=OH[:, step:step + 1], in1=MSEL[:, mb, :],
                            op0=OP.mult, op1=OP.add,
                        )
                    mprev = mcur

        if STAGE >= 4:
            # ============ phase B: M_loc1, mem_ret, combine, Wd ============
            with tc.tile_pool(name="pb", bufs=2) as pb, \
                 tc.tile_pool(name="psb", bufs=2, space="PSUM") as psb, \
                 tc.tile_pool(name="psw", bufs=2, space="PSUM") as psw:
                # M at segment 2c+1 = M + A0-part @ M + B0
                for mb in range(NB):
                    pm = psb.tile([P, HD], f32, tag="mm")
                    for kb in range(NB):
                        nc.tensor.matmul(
                            pm, lhsT=AT0[:, kb, mb * P:(mb + 1) * P], rhs=MSEL[:, kb, :],
                            start=(kb == 0), stop=False,
                        )
                    nc.tensor.matmul(pm, lhsT=ID, rhs=MSEL[:, mb, :], start=False, stop=False)
                    nc.tensor.matmul(pm, lhsT=ID, rhs=BT0[:, mb, :], start=False, stop=True)
                    nc.scalar.activation(MLOC1[:, mb, :], pm, AF.Copy)

                for j in range(SPC):
                    Mt = MSEL if j == 0 else MLOC1
                    for b in range(B):
                        i = bs_of(b, j)
                        st_i = pb.tile([P, NB, SEG], bf, tag="stp2", name=f"stp2_{i}")
                        nc.sync.dma_start(
                            out=st_i, in_=step_d[i].rearrange("kb p s -> p kb s"))
                        sq_i = sqT[i]
                        mtmp = pb.tile([P, NB, SEG], bf, tag="mt")
                        for mb in range(NB):
                            pm = psb.tile([P, SEG], f32, tag="mm")
                            for kb in range(NB):
                                nc.tensor.matmul(
                                    pm, lhsT=Mt[:, kb, mb * P:(mb + 1) * P],
                                    rhs=sq_i[:, kb, :],
                                    start=(kb == 0), stop=(kb == NB - 1),
                                )
                            nc.vector.scalar_tensor_tensor(
                                out=mtmp[:, mb, :], in0=pm, scalar=GC[:, mb:mb + 1],
                                in1=rcmbT[i],
                                op0=OP.mult, op1=OP.mult,
                            )
                        # out = (st_att + mem) @ Wd, both terms accumulated in PSUM
                        for sb in range(NB):
                            po = psw.tile([P, D], f32, tag="wd")
                            for mb in range(NB):
                                nc.tensor.matmul(
                                    po, lhsT=st_i[:, mb, sb * P:(sb + 1) * P],
                                    rhs=WD[:, mb, :],
                                    start=(mb == 0), stop=False,
                                )
                            for mb in range(NB):
                                nc.tensor.matmul(
                                    po, lhsT=mtmp[:, mb, sb * P:(sb + 1) * P],
                                    rhs=WD[:, mb, :],
                                    start=False, stop=(mb == NB - 1),
                                )
                            ob = pb.tile([P, D], f32, tag="ob")
                            nc.scalar.activation(ob, po, AF.Copy)
                            nc.sync.dma_start(
                                out=out_d[b, j, sb * P:(sb + 1) * P, :], in_=ob)

    nc.compile()
    return nc


def _prep_inputs(x, Wq, Wk, Wv, Wd, beta):
    """Host-side prep: transpose/cast/shard. Returns in_maps (list of 8 dicts)."""
    g = 1.0 / (1.0 + np.exp(-beta.astype(np.float64)))  # (H,)
    g = g.astype(np.float32)
    gcol = np.repeat(g, D).reshape(NB, P).T.copy()      # (P, NB): g[(kb*128+p)//64]
    omg = (1.0 - np.repeat(g, D)).reshape(NB, P).T.copy()

    def wprep(w):
        return np.ascontiguousarray(
            w.reshape(NB, P, w.shape[1]).astype(bf_np))

    wq_a, wk_a, wv_a = wprep(Wq), wprep(Wk), wprep(Wv)
    wd_a = wprep(Wd)
    cmask = np.triu(np.ones((P, P), np.float32)).astype(bf_np)
    ident = np.eye(P, dtype=np.float32).astype(bf_np)

    # x -> per-core transposed blocks: xt[b, j, kb, p, s] = x[b, (2c+j)*SEG+s, kb*P+p]
    xs = x.reshape(B, NSEG, SEG, DIN)
    in_maps = []
    for c in range(NC):
        xloc = xs[:, 2 * c:2 * c + 2]                        # (B, SPC, SEG, DIN)
        xt = xloc.transpose(0, 1, 3, 2)                      # (B, SPC, DIN, SEG)
        xt = np.ascontiguousarray(
            xt.reshape(B, SPC, NB, P, SEG).astype(bf_np))
        # AG1 global row for (t, b): rank t//2 contributes row (t%2)*B + b
        zmask = np.zeros((64, NC), np.float32)
        for jj in range(NC):
            tgt = 2 * c + (jj // B)
            bb = jj % B
            for t in range(NSEG):
                if t < tgt:
                    zmask[(t // 2) * BS + (t % 2) * B + bb, jj] = 1.0
        oh = np.zeros((P, NC), np.float32)
        if c >= 1:
            oh[:, c - 1] = 1.0
        in_maps.append({
            "xt": xt, "wq": wq_a, "wk": wk_a, "wv": wv_a, "wd": wd_a,
            "gcol": gcol, "omg": omg, "zmask": zmask, "oh": oh,
            "cmask": cmask, "ident": ident,
        })
    return in_maps


def kernel(x, Wq, Wk, Wv, Wd, beta, _trace=False):
    x = np.asarray(x, np.float32)
    in_maps = _prep_inputs(
        x, np.asarray(Wq, np.float32), np.asarray(Wk, np.float32),
        np.asarray(Wv, np.float32), np.asarray(Wd, np.float32),
        np.asarray(beta, np.float32))
    if "nc" not in _CACHE:
        _CACHE["nc"] = _build()
    nc = _CACHE["nc"]
    res = bass_utils.run_bass_kernel_spmd(
        nc, in_maps, core_ids=list(range(NC)), trace=_trace)
    _CACHE["last_results"] = res
    out = np.empty((B, L, D), np.float32)
    for c in range(NC):
        oc = res.results[c]["out"]                  # (B, SPC, SEG, D)
        out[:, 2 * c * SEG:(2 * c + 2) * SEG, :] = oc.reshape(B, SPC * SEG, D)
    return out


# revision 39
# speedup vs baseline: 19.3870x; 19.3870x over previous
"""Trainium2 Bass kernel for nn_MMHA_78039555768536.

Gated mix of per-segment causal softmax attention and a linear-attention
memory (delta rule, memory summed over batch per segment).

Strategy (8 cores): reformulate the memory recurrence as a linear matrix
recurrence  M_{t+1} = A_t M_t + B_t  with
    A_t = I - sum_b sk_b^T diag(1/d_b) sk_b   (symmetric A-part)
    B_t = sum_b sk_b^T v_b
    d_b = sk_b @ z_{b,t};  z is a prefix of column-sums of sk (M-independent)
Core c owns segments {2c, 2c+1} for all batches.  Two all-gathers:
 AG1: per-segment colsums of sk (for the z prefix)  [tiny]
 AG2: per-core pair composition (Abar^T, Bbar)      [1 MB bf16 per rank]
Then every core redundantly runs the 7-step pair chain and selects its own
prefix M via a per-core one-hot input (SPMD, no branches).

v2 schedule: all work that feeds AG2 (k-proj, colsums, AG1, z, v-proj,
A/B, compose) runs first; AG2 is issued, and the whole attention block
(q-proj, scores, softmax, attn@v, sq=elu1(q), memory-read denominators)
executes while the collective is in flight.  k is projected once
(transposed form) and PE-transposed into s-major for sk.  All partition
broadcasts are SBUF->SBUF gpsimd ops (no DRAM round-trips).
"""

import os
import sys

sys.path.insert(0, "/opt/trn_rl_repo")

STAGE = int(os.environ.get("KSTAGE", "9"))
SIMSAFE = int(os.environ.get("KSIMSAFE", "0"))  # CoreSim rejects PSUM re-accumulate

from contextlib import ExitStack

import numpy as np
import ml_dtypes

import concourse.bass as bass
import concourse.bacc as bacc
import concourse.tile as tile
from concourse import mybir
from concourse import bass_utils

B, L, DIN = 4, 8192, 512
H, D, SEG = 8, 64, 512
HD = H * D
NSEG = L // SEG          # 16
NC = 8                   # cores
SPC = NSEG // NC         # segments per core = 2
P = 128
NB = HD // P             # 4 blocks of 128
BS = B * SPC             # batch-segment units per core = 8

bf = mybir.dt.bfloat16
f32 = mybir.dt.float32
f8 = mybir.dt.float8e4
PM2 = mybir.MatmulPerfMode.DoubleRow
AF = mybir.ActivationFunctionType
OP = mybir.AluOpType
bf_np = ml_dtypes.bfloat16

_CACHE = {}


def _build():
    nc = bacc.Bacc(
        "TRN2",
        target_bir_lowering=False,
        debug=False,
        enable_asserts=False,
        num_devices=NC,
    )

    # ---------------- DRAM I/O ----------------
    xt_d = nc.dram_tensor("xt", [B, SPC, NB, P, SEG], bf, kind="ExternalInput").ap()
    wq_d = nc.dram_tensor("wq", [NB, P, HD], bf, kind="ExternalInput").ap()
    wk_d = nc.dram_tensor("wk", [NB, P, HD], bf, kind="ExternalInput").ap()
    wv_d = nc.dram_tensor("wv", [NB, P, HD], bf, kind="ExternalInput").ap()
    wd_d = nc.dram_tensor("wd", [NB, P, D], bf, kind="ExternalInput").ap()
    gcol_d = nc.dram_tensor("gcol", [P, NB], f32, kind="ExternalInput").ap()
    omg_d = nc.dram_tensor("omg", [P, NB], f32, kind="ExternalInput").ap()
    zmask_d = nc.dram_tensor("zmask", [64, NC], bf, kind="ExternalInput").ap()
    oh_d = nc.dram_tensor("oh", [P, NC], f32, kind="ExternalInput").ap()
    mask_d = nc.dram_tensor("cmask", [P, P], bf, kind="ExternalInput").ap()
    ident_d = nc.dram_tensor("ident", [P, P], bf, kind="ExternalInput").ap()
    dsc_d = nc.dram_tensor("dsc", [P, SPC], f32, kind="ExternalInput").ap()
    dscn_d = nc.dram_tensor("dscn", [P, SPC], f32, kind="ExternalInput").ap()
    out_d = nc.dram_tensor("out", [B, SPC, SEG, D], f32, kind="ExternalOutput").ap()

    with tile.TileContext(nc) as tc, ExitStack() as ctx:
        # ---------------- constant / DRAM pools ----------------
        const = ctx.enter_context(tc.tile_pool(name="const", bufs=1))
        dram = ctx.enter_context(tc.tile_pool(name="dram", bufs=1, space="DRAM"))
        keep = ctx.enter_context(tc.tile_pool(name="keep", bufs=BS))
        phb = ctx.enter_context(tc.tile_pool(name="phb", bufs=1))  # phase singles

        WQ = const.tile([P, NB, HD], bf)
        WK = const.tile([P, NB, HD], bf)
        WV = const.tile([P, NB, HD], bf)
        WD = const.tile([P, NB, D], bf)
        GC = const.tile([P, NB], f32)
        OMG = const.tile([P, NB], f32)
        ZM = const.tile([64, NC], bf)
        OH = const.tile([P, NC], f32)
        CM = const.tile([P, P], bf)
        ID = const.tile([P, P], bf)
        ONE = const.tile([P, 1], bf)
        ONER = const.tile([1, P], bf)   # ones row (PE partition-broadcast)
        DSC = const.tile([P, SPC], f32)
        DSCN = const.tile([P, SPC], f32)

        nc.sync.dma_start(out=WQ, in_=wq_d.rearrange("kb p n -> p kb n"))
        nc.sync.dma_start(out=WK, in_=wk_d.rearrange("kb p n -> p kb n"))
        nc.sync.dma_start(out=WV, in_=wv_d.rearrange("kb p n -> p kb n"))
        nc.sync.dma_start(out=WD, in_=wd_d.rearrange("kb p n -> p kb n"))
        nc.sync.dma_start(out=GC, in_=gcol_d)
        nc.sync.dma_start(out=OMG, in_=omg_d)
        nc.sync.dma_start(out=ZM, in_=zmask_d)
        nc.sync.dma_start(out=OH, in_=oh_d)
        nc.sync.dma_start(out=CM, in_=mask_d)
        nc.sync.dma_start(out=ID, in_=ident_d)
        nc.sync.dma_start(out=DSC, in_=dsc_d)
        nc.sync.dma_start(out=DSCN, in_=dscn_d)
        nc.vector.memset(ONE, 1.0)
        nc.vector.memset(ONER, 1.0)

        # collective bounce buffers
        cs_in = dram.tile([BS, HD], f32)
        cs_out = dram.tile([NC * BS, HD], f32)
        ab_in = dram.tile([2, HD, HD], bf)
        ab_out = dram.tile([NC, 2, HD, HD], bf, addr_space="Shared")
        step_d = dram.tile([BS, NB, P, SEG], bf)  # attention-term scratch

        # retained across phases (bufs=BS -> one slot per batch-segment)
        khT = [keep.tile([P, NB, SEG], bf, tag="kh", name=f"kh{i}") for i in range(BS)]
        skT = [keep.tile([P, NB, HD], bf, tag="sk", name=f"sk{i}") for i in range(BS)]
        sqT = skT   # sk dies before sq is born; share the slots
        vaT = [keep.tile([P, NB, H, D + 1], bf, tag="va", name=f"va{i}")
               for i in range(BS)]
        sk8T = [keep.tile([P, NB, HD], f8, tag="sk8", name=f"sk8_{i}")
                for i in range(BS)]
        va8T = [keep.tile([P, NB, HD], f8, tag="va8", name=f"va8_{i}")
                for i in range(BS)]
        rcmbT = [keep.tile([P, SEG], bf, tag="rcmb", name=f"rcmb{i}")
                 for i in range(BS)]

        # phase singles
        ZCOL = phb.tile([P, NB, BS], bf)    # column form for denominators
        AT0 = phb.tile([P, NB, HD], bf)     # segment-0 A-part (retained)
        BT0 = phb.tile([P, NB, HD], bf)
        MSEL = phb.tile([P, NB, HD], bf)    # selected M at segment 2c
        MLOC1 = BT0                         # M at segment 2c+1 reuses BT0's slot

        def bs_of(b, j):
            return j * B + b

        # ============ PHASE A1: k-proj (transposed), transpose->sk, colsums ====
        with tc.tile_pool(name="pa1", bufs=2) as pa1, \
             tc.tile_pool(name="ps1", bufs=2, space="PSUM") as ps1, \
             tc.tile_pool(name="pst", bufs=2, space="PSUM") as pst:
            for j in range(SPC):
                for b in range(B):
                    i = bs_of(b, j)
                    XT = pa1.tile([P, NB, SEG], bf, tag="xt")
                    nc.sync.dma_start(out=XT, in_=xt_d[b, j].rearrange("kb p s -> p kb s"))
                    kh_i = khT[i]
                    for mb in range(NB):
                        pk = ps1.tile([P, SEG], f32, tag="pk")
                        for kb in range(NB):
                            nc.tensor.matmul(
                                pk, lhsT=WK[:, kb, mb * P:(mb + 1) * P],
                                rhs=XT[:, kb, :],
                                start=(kb == 0), stop=(kb == NB - 1),
                            )
                        nc.scalar.activation(kh_i[:, mb, :], pk, AF.Copy)
                    # transpose kh -> s-major, fused elu1 into sk
                    sk_i = skT[i]
                    for sb in range(NB):
                        pt = pst.tile([P, HD], f32, tag="pt")
                        for mb in range(NB):
                            nc.tensor.matmul(
                                pt[:, mb * P:(mb + 1) * P],
                                lhsT=kh_i[:, mb, sb * P:(sb + 1) * P],
                                rhs=ID, start=True, stop=True,
                            )
                        # elu1(k) = max(k + 1, exp(min(k, 0)))
                        em = pa1.tile([P, HD], bf, tag="em")
                        nc.vector.tensor_scalar_min(em, pt, 0.0)
                        ee = pa1.tile([P, HD], bf, tag="ee")
                        nc.scalar.activation(ee, em, AF.Exp)
                        nc.vector.scalar_tensor_tensor(
                            out=sk_i[:, sb, :], in0=pt, scalar=1.0, in1=ee,
                            op0=OP.add, op1=OP.max,
                        )
                    nc.gpsimd.tensor_copy(sk8T[i], sk_i)
                    pc = ps1.tile([1, HD], f32, tag="pc")
                    for sb in range(NB):
                        nc.tensor.matmul(
                            pc, lhsT=ONE, rhs=sk_i[:, sb, :],
                            start=(sb == 0), stop=(sb == NB - 1),
                        )
                    cs_sb = pa1.tile([1, HD], f32, tag="cs")
                    nc.scalar.activation(cs_sb, pc, AF.Copy)
                    nc.sync.dma_start(out=cs_in[i:i + 1, :], in_=cs_sb)

        # ============ AG1: colsums ============
        nc.gpsimd.collective_compute(
            "AllGather", OP.bypass,
            replica_groups=[list(range(NC))],
            ins=[cs_in.opt()], outs=[cs_out.opt()],
        )

        # ============ v-proj (all units) + z prefix + A/B + compose ============
        with tc.tile_pool(name="pz", bufs=1) as pz, \
             tc.tile_pool(name="psz", bufs=1, space="PSUM") as psz, \
             tc.tile_pool(name="pskd", bufs=B) as pskd, \
             tc.tile_pool(name="pv2", bufs=2) as pv2, \
             tc.tile_pool(name="pab", bufs=1) as pab, \
             tc.tile_pool(name="ps2", bufs=2, space="PSUM") as ps2:
            # v-proj for all 8 units first: no dependency on AG1, so the PE
            # queue is not blocked behind the collective.
            for j in range(SPC):
                for b in range(B):
                    i = bs_of(b, j)
                    XT = pv2.tile([P, NB, SEG], bf, tag="xt")
                    nc.sync.dma_start(out=XT, in_=xt_d[b, j].rearrange("kb p s -> p kb s"))
                    va = vaT[i]
                    nc.vector.memset(va[:, :, :, D:D + 1], 1.0)
                    for sb in range(NB):
                        pv = ps2.tile([P, SEG], f32, tag="pp")
                        for kb in range(NB):
                            nc.tensor.matmul(
                                pv, lhsT=XT[:, kb, sb * P:(sb + 1) * P],
                                rhs=WV[:, kb, :],
                                start=(kb == 0), stop=(kb == NB - 1),
                            )
                        nc.vector.tensor_copy(
                            va[:, sb, :, 0:D], pv.rearrange("p (h d) -> p h d", h=H)
                        )
                    nc.gpsimd.tensor_copy(
                        va8T[i].rearrange("p kb (h d) -> p kb h d", h=H),
                        va[:, :, :, 0:D])

            zrow16 = []
            with tc.tile_pool(name="pzz", bufs=1) as pzz:
                Z = pzz.tile([NC * BS, HD], bf, tag="z")
                nc.gpsimd.dma_start(out=Z, in_=cs_out)
                # per-unit z rows, each in its own partition-0 tile (ISA
                # partition_broadcast requires an aligned source partition)
                for i in range(BS):
                    zpi = psz.tile([1, HD], f32, tag="zpi", bufs=2)
                    nc.tensor.matmul(zpi, lhsT=ZM[:, i:i + 1], rhs=Z,
                                     start=True, stop=True)
                    zr = pz.tile([1, HD], bf, tag="zr16", name=f"zr16_{i}", bufs=BS)
                    nc.scalar.activation(zr, zpi, AF.Copy, bias=1.0 / D)
                    zrow16.append(zr)
                for kb in range(NB):
                    zc = psz.tile([P, BS], f32, tag="zc")
                    nc.tensor.matmul(zc, lhsT=Z[:, kb * P:(kb + 1) * P], rhs=ZM,
                                     start=True, stop=True)
                    nc.scalar.activation(ZCOL[:, kb, :], zc, AF.Copy, bias=1.0 / D)

            at1 = bt1 = None
            for j in range(SPC):
                skd = [None] * B
                for b in range(B):
                    i = bs_of(b, j)
                    # --- d and sk/d (elementwise on gpsimd, free pre-AG2) ---
                    sk_i = skT[i]
                    sd = pskd.tile([P, NB, HD], f8, tag="skd")
                    skd[b] = sd
                    dcol = pv2.tile([P, NB], f32, tag="d")
                    rcd = pv2.tile([P, NB], f32, tag="rcd")
                    rcd2 = pv2.tile([P, NB], f32, tag="rcd2")
                    zbp = pv2.tile([P, HD], bf, tag="zbp")
                    nc.gpsimd.partition_broadcast(zbp, zrow16[i])
                    for sb in range(NB):
                        jnk = pv2.tile([P, HD], bf, tag="jnk")
                        nc.gpsimd.tensor_mul(jnk, sk_i[:, sb, :], zbp)
                        nc.vector.tensor_reduce(
                            out=dcol[:, sb:sb + 1], in_=jnk,
                            axis=mybir.AxisListType.X, op=OP.add,
                        )
                    nc.vector.reciprocal(rcd, dcol)
                    # scale by the host d-estimate so skd fits fp8e4 range
                    nc.vector.tensor_scalar_mul(rcd2, rcd, DSC[:, j:j + 1])
                    for sb in range(NB):
                        nc.vector.tensor_scalar_mul(
                            sd[:, sb, :], sk_i[:, sb, :], rcd2[:, sb:sb + 1]
                        )

                # --- A_t, B_t for this segment (sum over batches) ---
                at_t = pab.tile([P, NB, HD], bf, tag="at", name=f"at{j}") if j > 0 else AT0
                bt_t = pab.tile([P, NB, HD], bf, tag="bt", name=f"bt{j}") if j > 0 else BT0
                npair = NB // 2
                for mb in range(NB):
                    pA = ps2.tile([P, HD], f32, tag="pp")
                    n = 0
                    for b in range(B):
                        for sp in range(npair):
                            sb = 2 * sp
                            nc.tensor.matmul(
                                pA,
                                lhsT=sk8T[bs_of(b, j)][:, sb:sb + 2, mb * P:(mb + 1) * P],
                                rhs=skd[b][:, sb:sb + 2, :],
                                start=(n == 0), stop=(n == B * npair - 1),
                                perf_mode=PM2,
                            )
                            n += 1
                    # negate and undo the fp8 d-scale: A-part = -K/dbar
                    nc.scalar.activation(at_t[:, mb, :], pA, AF.Copy,
                                         scale=DSCN[:, j:j + 1])
                for mb in range(NB):
                    pB = ps2.tile([P, HD], f32, tag="pp")
                    n = 0
                    for b in range(B):
                        for sp in range(npair):
                            sb = 2 * sp
                            nc.tensor.matmul(
                                pB,
                                lhsT=sk8T[bs_of(b, j)][:, sb:sb + 2, mb * P:(mb + 1) * P],
                                rhs=va8T[bs_of(b, j)][:, sb:sb + 2, :],
                                start=(n == 0), stop=(n == B * npair - 1),
                                perf_mode=PM2,
                            )
                            n += 1
                    nc.scalar.activation(bt_t[:, mb, :], pB, AF.Copy)
                if j > 0:
                    at1, bt1 = at_t, bt_t

            # --- pair composition: abA = Abar^T = A0 A1 + A0 + A1 ; abB = Bbar ---
            abA = pab.tile([P, NB, HD], bf, tag="abA")
            abB = pab.tile([P, NB, HD], bf, tag="abB")
            for mb in range(NB):
                pA = ps2.tile([P, HD], f32, tag="pp")
                for kb in range(NB):
                    nc.tensor.matmul(
                        pA, lhsT=AT0[:, kb, mb * P:(mb + 1) * P], rhs=at1[:, kb, :],
                        start=(kb == 0), stop=False,
                    )
                nc.tensor.matmul(pA, lhsT=ID, rhs=AT0[:, mb, :], start=False, stop=False)
                nc.tensor.matmul(pA, lhsT=ID, rhs=at1[:, mb, :], start=False, stop=True)
                nc.scalar.activation(abA[:, mb, :], pA, AF.Copy)
            for mb in range(NB):
                pB = ps2.tile([P, HD], f32, tag="pp")
                for kb in range(NB):
                    nc.tensor.matmul(
                        pB, lhsT=at1[:, kb, mb * P:(mb + 1) * P], rhs=BT0[:, kb, :],
                        start=(kb == 0), stop=False,
                    )
                nc.tensor.matmul(pB, lhsT=ID, rhs=BT0[:, mb, :], start=False, stop=False)
                nc.tensor.matmul(pB, lhsT=ID, rhs=bt1[:, mb, :], start=False, stop=True)
                nc.scalar.activation(abB[:, mb, :], pB, AF.Copy)
            nc.sync.dma_start(out=ab_in[0].rearrange("(kb p) n -> p kb n", p=P), in_=abA)
            nc.sync.dma_start(out=ab_in[1].rearrange("(kb p) n -> p kb n", p=P), in_=abB)

        # ============ AG2: pair compositions (overlapped with attention) ======
        nc.gpsimd.collective_compute(
            "AllGather", OP.bypass,
            replica_groups=[list(range(NC))],
            ins=[ab_in.opt()], outs=[ab_out.opt()],
        )

        if STAGE >= 2:
            # ============ WINDOW: q-proj, sq, attention, mem-read denominators
            with tc.tile_pool(name="pw1", bufs=2) as pw1, \
                 tc.tile_pool(name="pw", bufs=3) as pw, \
                 tc.tile_pool(name="psq", bufs=2, space="PSUM") as psq, \
                 tc.tile_pool(name="psc", bufs=2, space="PSUM") as psc, \
                 tc.tile_pool(name="psa", bufs=2, space="PSUM") as psa, \
                 tc.tile_pool(name="psd", bufs=1, space="PSUM") as psd:
                for j in range(SPC):
                    for b in range(B):
                        i = bs_of(b, j)
                        XT = pw1.tile([P, NB, SEG], bf, tag="xt")
                        nc.sync.dma_start(out=XT, in_=xt_d[b, j].rearrange("kb p s -> p kb s"))
                        # --- qT (hd on partitions) + sq = elu1(q) ---
                        qh = pw1.tile([P, NB, SEG], bf, tag="qh")
                        sq_i = sqT[i]
                        for mb in range(NB):
                            pq = psq.tile([P, SEG], f32, tag="pp")
                            for kb in range(NB):
                                nc.tensor.matmul(
                                    pq, lhsT=WQ[:, kb, mb * P:(mb + 1) * P],
                                    rhs=XT[:, kb, :],
                                    start=(kb == 0), stop=(kb == NB - 1),
                                )
                            nc.scalar.activation(qh[:, mb, :], pq, AF.Copy)
                            em = pw1.tile([P, SEG], bf, tag="em")
                            nc.vector.tensor_scalar_min(em, pq, 0.0)
                            ee = pw1.tile([P, SEG], bf, tag="ee")
                            nc.scalar.activation(ee, em, AF.Exp)
                            nc.vector.scalar_tensor_tensor(
                                out=sq_i[:, mb, :], in0=pq, scalar=1.0, in1=ee,
                                op0=OP.add, op1=OP.max,
                            )
                        # --- memory-read denominator -> broadcast reciprocal ---
                        pd = psd.tile([1, SEG], f32, tag="dn")
                        for kb in range(NB):
                            nc.tensor.matmul(
                                pd, lhsT=ZCOL[:, kb, i:i + 1], rhs=sq_i[:, kb, :],
                                start=(kb == 0), stop=(kb == NB - 1),
                            )
                        rcm = pw.tile([1, SEG], bf, tag="rcm")
                        with nc.allow_low_precision(reason="bf16 memread recip"):
                            nc.vector.reciprocal(rcm, pd)
                        pbm = psd.tile([P, SEG], f32, tag="bc")
                        nc.tensor.matmul(pbm, lhsT=ONER, rhs=rcm,
                                         start=True, stop=True)
                        nc.vector.tensor_copy(rcmbT[i], pbm)

                        # --- attention (no Pool-engine ops: queue is blocked
                        #     behind AG2; broadcast reciprocals via PE) ---
                        kh_i = khT[i]
                        va = vaT[i]
                        st_i = pw1.tile([P, NB, SEG], bf, tag="stp")
                        for h in range(H):
                            hb, ho = h // 2, (h % 2) * 64
                            pat = psa.tile([D + 1, SEG], f32, tag="at")
                            for kb in range(NB):
                                q0 = kb * P
                                qf = SEG - q0
                                ps_ = psc.tile([P, SEG], f32, tag="sc")
                                nc.tensor.matmul(
                                    ps_[:, 0:qf],
                                    lhsT=kh_i[ho:ho + 64, hb, q0:q0 + P],
                                    rhs=qh[ho:ho + 64, hb, q0:SEG],
                                    start=True, stop=True,
                                )
                                wt = pw.tile([P, SEG], bf, tag="wt")
                                nc.scalar.activation(wt[:, 0:qf], ps_[:, 0:qf], AF.Exp,
                                                     scale=0.125)
                                # causal mask on the diagonal 128x128 block
                                nc.vector.tensor_mul(wt[:, 0:P], wt[:, 0:P], CM)
                                nc.tensor.matmul(
                                    pat[:, q0:SEG],
                                    lhsT=va[:, kb, h, :],
                                    rhs=wt[:, 0:qf],
                                    start=(kb == 0), stop=(kb == NB - 1),
                                )
                            rca = pw.tile([1, SEG], bf, tag="rca")
                            with nc.allow_low_precision(reason="bf16 softmax recip"):
                                nc.vector.reciprocal(rca, pat[D:D + 1, :])
                            # broadcast 1/den across partitions via PE (Pool
                            # queue is blocked behind AG2)
                            pbc = psd.tile([D, SEG], f32, tag="bc")
                            nc.tensor.matmul(pbc, lhsT=ONER[:, 0:D], rhs=rca,
                                             start=True, stop=True)
                            # DVE may read only one PSUM operand: move the
                            # numerator to SBUF (folding the (1-g) gate) first
                            stp = pw.tile([D, SEG], bf, tag="stpre")
                            nc.scalar.activation(stp, pat[0:D, :], AF.Copy,
                                                 scale=OMG[ho:ho + 64, hb:hb + 1])
                            nc.vector.tensor_mul(st_i[ho:ho + 64, hb, :], stp, pbc)
                        nc.sync.dma_start(
                            out=step_d[i].rearrange("kb p s -> p kb s"), in_=st_i)

        if STAGE >= 3:
            # ============ chain + select ============
            nc.vector.memset(MSEL, 0.0)
            with tc.tile_pool(name="pch", bufs=2) as pch, \
                 tc.tile_pool(name="psch", bufs=NB, space="PSUM") as psch:
                pM = [psch.tile([P, HD], f32, tag="ch", name=f"chain{i}") for i in range(NB)]
                mprev = None
                for step in range(NC - 1):
                    cA = pch.tile([P, NB, HD], bf, tag="cA")
                    cB = pch.tile([P, NB, HD], bf, tag="cB")
                    nc.sync.dma_start(
                        out=cA, in_=ab_out[step, 0].rearrange("(kb p) n -> p kb n", p=P))
                    nc.sync.dma_start(
                        out=cB, in_=ab_out[step, 1].rearrange("(kb p) n -> p kb n", p=P))
                    mcur = pch.tile([P, NB, HD], bf, tag="mc")
                    for mb in range(NB):
                        if step == 0:
                            nc.tensor.matmul(pM[mb], lhsT=ID, rhs=cB[:, mb, :],
                                             start=True, stop=True)
                        elif SIMSAFE:
                            for kb in range(NB):
                                nc.tensor.matmul(
                                    pM[mb], lhsT=cA[:, kb, mb * P:(mb + 1) * P],
                                    rhs=mprev[:, kb, :],
                                    start=(kb == 0), stop=False,
                                )
                            nc.tensor.matmul(pM[mb], lhsT=ID, rhs=mprev[:, mb, :],
                                             start=False, stop=False)
                            nc.tensor.matmul(pM[mb], lhsT=ID, rhs=cB[:, mb, :],
                                             start=False, stop=True)
                        else:
                            for kb in range(NB):
                                nc.tensor.matmul(
                                    pM[mb], lhsT=cA[:, kb, mb * P:(mb + 1) * P],
                                    rhs=mprev[:, kb, :],
                                    start=False, stop=False,
                                )
                            nc.tensor.matmul(pM[mb], lhsT=ID, rhs=cB[:, mb, :],
                                             start=False, stop=True)
                        # split psum->sbuf copies across ACT and DVE: all four
                        # blocks gate the next chain step
                        if mb < 2:
                            nc.scalar.activation(mcur[:, mb, :], pM[mb], AF.Copy)
                        else:
                            nc.vector.tensor_copy(mcur[:, mb, :], pM[mb])
                        nc.vector.scalar_tensor_tensor(
                            out=MSEL[:, mb, :], in0=mcur[:, mb, :],
                            scalar=OH[:, step:step + 1], in1=MSEL[:, mb, :],
                            op0=OP.mult, op1=OP.add,
                        )
                    mprev = mcur

        if STAGE >= 4:
            # ============ phase B: M_loc1, mem_ret, combine, Wd ============
            with tc.tile_pool(name="pb", bufs=2) as pb, \
                 tc.tile_pool(name="psb", bufs=2, space="PSUM") as psb, \
                 tc.tile_pool(name="psw", bufs=2, space="PSUM") as psw:
                # M at segment 2c+1 = M + A0-part @ M + B0
                for mb in range(NB):
                    pm = psb.tile([P, HD], f32, tag="mm")
                    for kb in range(NB):
                        nc.tensor.matmul(
                            pm, lhsT=AT0[:, kb, mb * P:(mb + 1) * P], rhs=MSEL[:, kb, :],
                            start=(kb == 0), stop=False,
                        )
                    nc.tensor.matmul(pm, lhsT=ID, rhs=MSEL[:, mb, :], start=False, stop=False)
                    nc.tensor.matmul(pm, lhsT=ID, rhs=BT0[:, mb, :], start=False, stop=True)
                    nc.scalar.activation(MLOC1[:, mb, :], pm, AF.Copy)

                for j in range(SPC):
                    Mt = MSEL if j == 0 else MLOC1
                    for b in range(B):
                        i = bs_of(b, j)
                        st_i = pb.tile([P, NB, SEG], bf, tag="stp2", name=f"stp2_{i}")
                        nc.sync.dma_start(
                            out=st_i, in_=step_d[i].rearrange("kb p s -> p kb s"))
                        sq_i = sqT[i]
                        mtmp = pb.tile([P, NB, SEG], bf, tag="mt")
                        for mb in range(NB):
                            pm = psb.tile([P, SEG], f32, tag="mm")
                            for kb in range(NB):
                                nc.tensor.matmul(
                                    pm, lhsT=Mt[:, kb, mb * P:(mb + 1) * P],
                                    rhs=sq_i[:, kb, :],
                                    start=(kb == 0), stop=(kb == NB - 1),
                                )
                            nc.vector.scalar_tensor_tensor(
                                out=mtmp[:, mb, :], in0=pm, scalar=GC[:, mb:mb + 1],
                                in1=rcmbT[i],
                                op0=OP.mult, op1=OP.mult,
                            )
                        # out = (st_att + mem) @ Wd, both terms accumulated in PSUM
                        for sb in range(NB):
                            po = psw.tile([P, D], f32, tag="wd")
                            for mb in range(NB):
                                nc.tensor.matmul(
                                    po, lhsT=st_i[:, mb, sb * P:(sb + 1) * P],
                                    rhs=WD[:, mb, :],
                                    start=(mb == 0), stop=False,
                                )
                            for mb in range(NB):
                                nc.tensor.matmul(
                                    po, lhsT=mtmp[:, mb, sb * P:(sb + 1) * P],
                                    rhs=WD[:, mb, :],
                                    start=False, stop=(mb == NB - 1),
                                )
                            ob = pb.tile([P, D], f32, tag="ob")
                            nc.scalar.activation(ob, po, AF.Copy)
                            nc.sync.dma_start(
                                out=out_d[b, j, sb * P:(sb + 1) * P, :], in_=ob)

    nc.compile()
    return nc


def _prep_inputs(x, Wq, Wk, Wv, Wd, beta):
    """Host-side prep: transpose/cast/shard. Returns in_maps (list of 8 dicts)."""
    g = 1.0 / (1.0 + np.exp(-beta.astype(np.float64)))  # (H,)
    g = g.astype(np.float32)
    gcol = np.repeat(g, D).reshape(NB, P).T.copy()      # (P, NB): g[(kb*128+p)//64]
    omg = (1.0 - np.repeat(g, D)).reshape(NB, P).T.copy()

    def wprep(w):
        return np.ascontiguousarray(
            w.reshape(NB, P, w.shape[1]).astype(bf_np))

    wq_a, wk_a, wv_a = wprep(Wq), wprep(Wk), wprep(Wv)
    wd_a = wprep(Wd)
    cmask = np.triu(np.ones((P, P), np.float32)).astype(bf_np)
    ident = np.eye(P, dtype=np.float32).astype(bf_np)

    # Statistical estimate dbar(t) of the delta-rule denominator per segment,
    # used to scale skd into fp8e4 range on device (exact unscale after).
    # k col j ~ N(0, ||Wk[:,j]||^2); E[elu1(N(0,s))] in closed form.
    import math
    sig = np.sqrt((Wk.astype(np.float64) ** 2).sum(axis=0))          # (HD,)
    f = (np.exp(sig * sig / 2.0)
         * np.array([0.5 * math.erfc(s / math.sqrt(2.0)) for s in sig])
         + sig / math.sqrt(2.0 * math.pi) + 0.5)
    dbar = np.array([
        float((f * (1.0 / D + t * SEG * f)).sum()) for t in range(NSEG)
    ])

    # x -> per-core transposed blocks: xt[b, j, kb, p, s] = x[b, (2c+j)*SEG+s, kb*P+p]
    xs = x.reshape(B, NSEG, SEG, DIN)
    in_maps = []
    for c in range(NC):
        xloc = xs[:, 2 * c:2 * c + 2]                        # (B, SPC, SEG, DIN)
        xt = xloc.transpose(0, 1, 3, 2)                      # (B, SPC, DIN, SEG)
        xt = np.ascontiguousarray(
            xt.reshape(B, SPC, NB, P, SEG).astype(bf_np))
        # AG1 global row for (t, b): rank t//2 contributes row (t%2)*B + b
        zmask = np.zeros((64, NC), np.float32)
        for jj in range(NC):
            tgt = 2 * c + (jj // B)
            bb = jj % B
            for t in range(NSEG):
                if t < tgt:
                    zmask[(t // 2) * BS + (t % 2) * B + bb, jj] = 1.0
        oh = np.zeros((P, NC), np.float32)
        if c >= 1:
            oh[:, c - 1] = 1.0
        dsc = np.broadcast_to(dbar[2 * c:2 * c + 2], (P, SPC)).astype(np.float32).copy()
        dscn = (-1.0 / dsc).astype(np.float32)
        in_maps.append({
            "xt": xt, "wq": wq_a, "wk": wk_a, "wv": wv_a, "wd": wd_a,
            "gcol": gcol, "omg": omg, "zmask": zmask.astype(bf_np), "oh": oh,
            "cmask": cmask, "ident": ident, "dsc": dsc, "dscn": dscn,
        })
    return in_maps


def kernel(x, Wq, Wk, Wv, Wd, beta, _trace=False):
    x = np.asarray(x, np.float32)
    in_maps = _prep_inputs(
        x, np.asarray(Wq, np.float32), np.asarray(Wk, np.float32),
        np.asarray(Wv, np.float32), np.asarray(Wd, np.float32),
        np.asarray(beta, np.float32))
    if "nc" not in _CACHE:
        _CACHE["nc"] = _build()
    nc = _CACHE["nc"]
    res = bass_utils.run_bass_kernel_spmd(
        nc, in_maps, core_ids=list(range(NC)), trace=_trace)
    _CACHE["last_results"] = res
    out = np.empty((B, L, D), np.float32)
    for c in range(NC):
        oc = res.results[c]["out"]                  # (B, SPC, SEG, D)
        out[:, 2 * c * SEG:(2 * c + 2) * SEG, :] = oc.reshape(B, SPC * SEG, D)
    return out
